# revision 12
# baseline (speedup 1.0000x reference)
"""MultiHeadAttention (B=4, S=2048, D=1024, H=16, causal) on 8 TRN2 NeuronCores.

Sharding: tensor-parallel over heads across all 8 cores (2 heads/core, all 4
batches processed locally; identical SPMD control flow on every core). After
attention, two 8-core AllToAlls (batches 0-1, then 2-3, the first overlapped
with compute) redistribute the transposed attention outputs so each core runs
the output projection for 1/8 of the (batch, seq) rows. Host side only
slices/transposes/casts inputs and concatenates outputs.

Per-core pipeline (all matmuls bf16 with f32 PSUM accumulation):
  - x arrives host-transposed as x^T [D, B*S] in bf16; K^T, Q^T, V^T
    ([head-cols, seq]) via w-stationary matmuls with per-partition bias on the
    ScalarE eviction; V^T is PE-transposed to natural V with a ones column
    appended per head so the PV matmul also produces the softmax denominator.
  - Scores are computed transposed ([k, q] = K @ Q^T) in 512-wide chunks
    aligned to absolute q columns, exp on ScalarE with the 1/sqrt(dk) scale
    folded in (no max subtraction needed: |scores| <~ 2), causal mask applied
    as a 0/1 upper-triangular multiply on diagonal tiles only; fully-masked
    tiles are skipped entirely.
  - PV: [V|1] is the stationary operand (one weight load per k-strip), exp
    chunks stream through, accumulating out^T [65, q] in PSUM; PV emission
    runs one k-strip behind scores/exp so the in-order PE queue never
    head-of-line blocks on ScalarE.
  - out^T is transposed back per q-tile and normalized by the ones column
    ([p,1] broadcast) into the concatenated attention output.
"""

import sys

if "/opt/trn_rl_repo" not in sys.path:
    sys.path.insert(0, "/opt/trn_rl_repo")

from contextlib import ExitStack

import ml_dtypes
import numpy as np

import concourse.bacc as bacc
import concourse.bass as bass
import concourse.mybir as mybir
import concourse.tile as tile
from concourse.bass_utils import run_bass_kernel_spmd
from concourse.masks import make_identity, make_upper_triangular

N_CORES = 8
B = 4
S = 2048
D = 1024
H_TOT = 16
DK = 64
H_LOC = H_TOT // N_CORES  # 2 heads per core
HC = H_LOC * DK  # 128 head-cols per core
ST = S // 128  # 16 seq tiles per batch
DC = D // 128  # 8 d_model chunks
BQ = (B * S) // N_CORES  # 1024 (batch,seq) rows per core after AllToAll

F32 = mybir.dt.float32
BF16 = mybir.dt.bfloat16
BF16_NP = ml_dtypes.bfloat16


def _bcast(handle, rows, cols):
    """AP reading a [1, cols] DRAM tensor broadcast over `rows` partitions."""
    return bass.AP(tensor=handle, offset=0, ap=[[0, rows], [1, cols]])


def build_program():
    nc = bacc.Bacc("TRN2", target_bir_lowering=False, debug=False,
                   num_devices=N_CORES)

    xt = nc.declare_dram_parameter("xt", [D, B * S], BF16, isOutput=False)
    wq = nc.declare_dram_parameter("wq", [D, HC], BF16, isOutput=False)
    wk = nc.declare_dram_parameter("wk", [D, HC], BF16, isOutput=False)
    wv = nc.declare_dram_parameter("wv", [D, HC], BF16, isOutput=False)
    bq = nc.declare_dram_parameter("bq", [HC, 1], F32, isOutput=False)
    bk = nc.declare_dram_parameter("bk", [HC, 1], F32, isOutput=False)
    bv = nc.declare_dram_parameter("bv", [HC, 1], F32, isOutput=False)
    wo = nc.declare_dram_parameter("wo", [D, D], BF16, isOutput=False)
    bo = nc.declare_dram_parameter("bo", [1, D], F32, isOutput=False)
    out = nc.declare_dram_parameter("out", [BQ, D], F32, isOutput=True)

    with ExitStack() as ctx:
        tc = ctx.enter_context(tile.TileContext(nc))

        consts = ctx.enter_context(tc.tile_pool(name="consts", bufs=1))
        wpool = ctx.enter_context(tc.tile_pool(name="wpool", bufs=1))
        xtp = ctx.enter_context(tc.tile_pool(name="xtp", bufs=1))
        kqv = ctx.enter_context(tc.tile_pool(name="kqv", bufs=2))
        epool = ctx.enter_context(tc.tile_pool(name="epool", bufs=8))
        aopool = ctx.enter_context(tc.tile_pool(name="aopool", bufs=2))
        rpool = ctx.enter_context(tc.tile_pool(name="rpool", bufs=4))
        opool = ctx.enter_context(tc.tile_pool(name="opool", bufs=2))
        ps_s = ctx.enter_context(tc.tile_pool(name="ps_s", bufs=1, space="PSUM"))
        ps_b = ctx.enter_context(tc.tile_pool(name="ps_b", bufs=3, space="PSUM"))
        ps_o = ctx.enter_context(tc.tile_pool(name="ps_o", bufs=1, space="PSUM"))
        dram = ctx.enter_context(tc.tile_pool(name="dram", bufs=1, space="DRAM"))

        # four quarter-exchanges (one per batch); only the last is exposed
        in_b = [dram.tile([N_CORES * 128, 256], BF16, tag=f"in_b{i}",
                          name=f"in_b{i}") for i in range(B)]
        out_b = [dram.tile([N_CORES * 128, 256], BF16, tag=f"out_b{i}",
                           name=f"out_b{i}") for i in range(B)]

        # --- constants ---
        ident = consts.tile([128, 128], F32)
        make_identity(nc, ident)
        ident_bf = consts.tile([128, 128], BF16)
        make_identity(nc, ident_bf)
        triu = consts.tile([128, 128], BF16)
        make_upper_triangular(nc, triu, 1.0, diag=True)
        bq_sb = consts.tile([HC, 1], F32)
        nc.sync.dma_start(out=bq_sb, in_=bq[:, :])
        bk_sb = consts.tile([HC, 1], F32)
        nc.sync.dma_start(out=bk_sb, in_=bk[:, :])
        bv_sb = consts.tile([HC, 1], F32)
        nc.sync.dma_start(out=bv_sb, in_=bv[:, :])
        bo_sb = consts.tile([128, D], F32)
        nc.sync.dma_start(out=bo_sb, in_=_bcast(bo, 128, D))

        # --- weights (already bf16) ---
        wq_sb = wpool.tile([128, DC, HC], BF16, tag="wq_sb")
        nc.sync.dma_start(out=wq_sb, in_=wq.rearrange("(c p) m -> p c m", p=128))
        wk_sb = wpool.tile([128, DC, HC], BF16, tag="wk_sb")
        nc.sync.dma_start(out=wk_sb, in_=wk.rearrange("(c p) m -> p c m", p=128))
        wv_sb = wpool.tile([128, DC, HC], BF16, tag="wv_sb")
        nc.sync.dma_start(out=wv_sb, in_=wv.rearrange("(c p) m -> p c m", p=128))
        wo_sb = wpool.tile([128, DC, D], BF16, tag="wo_sb")
        nc.sync.dma_start(out=wo_sb, in_=wo.rearrange("(c p) m -> p c m", p=128))

        for b in range(B):
            # x^T slab for this batch (bf16, host-prepared)
            xT = xtp.tile([128, DC, S], BF16, tag="xT")
            for c in range(DC):
                nc.sync.dma_start(
                    out=xT[:, c, :],
                    in_=xt[c * 128:(c + 1) * 128, b * S:(b + 1) * S])

            # K^T, Q^T, V^T: [HC, S] with per-partition bias on eviction
            kt = kqv.tile([HC, S], BF16, tag="kt")
            qt_ = kqv.tile([HC, S], BF16, tag="qt")
            vt = kqv.tile([HC, S], BF16, tag="vt")
            for dst, w_sb, b_sb in ((kt, wk_sb, bk_sb), (qt_, wq_sb, bq_sb),
                                    (vt, wv_sb, bv_sb)):
                for s4 in range(S // 512):
                    p = ps_b.tile([128, 512], F32, tag="ps_b")
                    for c in range(DC):
                        nc.tensor.matmul(p, lhsT=w_sb[:, c, :],
                                         rhs=xT[:, c, s4 * 512:(s4 + 1) * 512],
                                         start=(c == 0), stop=(c == DC - 1))
                    nc.vector.tensor_scalar_add(
                        dst[:, s4 * 512:(s4 + 1) * 512], p, b_sb)

            # V natural [seq, head, 64|1] via PE transpose of V^T
            vsb = kqv.tile([128, ST, H_LOC * 65], BF16, tag="vsb")
            ones_view = vsb.rearrange("p s (h o) -> p s h o", o=65)[:, :, :, 64:65]
            nc.vector.memset(ones_view, 1.0)
            for st in range(ST):
                pt = ps_s.tile([128, 128], BF16, tag="ps_t")
                nc.tensor.transpose(pt, vt[:, st * 128:st * 128 + 128], ident_bf)
                v_view = vsb.rearrange("p s (h o) -> p s h o", o=65)[:, st, :, 0:64]
                nc.vector.tensor_copy(v_view,
                                      pt.rearrange("p (h d) -> p h d", d=DK))

            # attention
            aos = aopool.tile([128, ST, HC], F32, tag="ao")
            for h in range(H_LOC):
                # out^T accumulator: [65, q]; each 512-col bank holds 4 q-tiles
                po = ps_o.tile([65, ST * 128], F32, tag="ps_o")

                def emit_pv(chunks):
                    # PV one strip behind scores/exp: [V|1] stationary (one
                    # LDWEIGHTS per strip), exp chunks stream as the moving
                    # operand. start clears has_written for the whole PSUM
                    # bank, so only the j==0 chunks (which each cover exactly
                    # one bank) set it.
                    for ec, j0, c0, cw in chunks:
                        a = c0 // 512
                        nc.tensor.matmul(
                            po[:, c0:c0 + cw],
                            lhsT=vsb[:, j0, h * 65:(h + 1) * 65],
                            rhs=ec[:, 0:cw],
                            start=(j0 == 0), stop=(j0 == 4 * a + 3),
                            skip_group_check=True)

                prev = None
                for j in range(ST):
                    cur = []
                    for a in range(j // 4, 4):
                        c0 = max(512 * a, 128 * j)
                        cw = 512 * (a + 1) - c0
                        ps = ps_b.tile([128, 512], F32, tag="ps_b")
                        nc.tensor.matmul(
                            ps[:, 0:cw],
                            lhsT=kt[h * DK:(h + 1) * DK, j * 128:j * 128 + 128],
                            rhs=qt_[h * DK:(h + 1) * DK, c0:c0 + cw],
                            start=True, stop=True)
                        ec = epool.tile([128, 512], BF16, tag="et")
                        nc.scalar.activation(ec[:, 0:cw], ps[:, 0:cw],
                                             mybir.ActivationFunctionType.Exp,
                                             scale=1.0 / np.sqrt(DK))
                        if c0 == 128 * j:
                            # first 128 cols of the strip are the diagonal
                            nc.vector.tensor_mul(ec[:, 0:128], ec[:, 0:128],
                                                 triu)
                        cur.append((ec, j, c0, cw))
                    if prev:
                        emit_pv(prev)
                    prev = cur
                emit_pv(prev)

                # evict out^T, transpose each q-tile back, normalize
                poT = aopool.tile([65, ST * 128], F32, tag="poT")
                nc.vector.tensor_copy(poT, po)
                for g in range(ST):
                    pt = ps_s.tile([128, 128], F32, tag="ps_t")
                    nc.tensor.transpose(pt[:, 0:65],
                                        poT[:, g * 128:g * 128 + 128],
                                        ident[0:65, 0:65])
                    rcp = rpool.tile([128, 1], F32, tag="rcp")
                    nc.vector.reciprocal(rcp, pt[:, 64:65])
                    nc.vector.tensor_scalar_mul(
                        aos[:, g, h * DK:(h + 1) * DK], pt[:, 0:64], rcp)

            # transpose [q, dm] -> [dm, q], cast bf16, ship to bounce buffer
            for g in range(ST):
                pt = ps_s.tile([128, 128], F32, tag="ps_t")
                nc.tensor.transpose(pt, aos[:, g, :], ident)
                aoT = aopool.tile([128, 128], BF16, tag="aoT")
                nc.vector.tensor_copy(aoT, pt)
                shard = g // 2
                col = (g % 2) * 128
                nc.sync.dma_start(
                    out=in_b[b][shard * 128:(shard + 1) * 128, col:col + 128],
                    in_=aoT)

            nc.gpsimd.collective_compute(
                "AllToAll", mybir.AluOpType.bypass,
                replica_groups=[list(range(N_CORES))],
                ins=[in_b[b].opt()], outs=[out_b[b].opt()])

        # full attn_out^T for my 1/8 of (b, q): rows = my two half-batches
        aT = wpool.tile([128, DC, BQ], BF16, tag="aT")
        for c in range(DC):
            for k in range(B):
                nc.sync.dma_start(
                    out=aT[:, c, k * 256:(k + 1) * 256],
                    in_=out_b[k][c * 128:(c + 1) * 128, :])

        # --- output projection: out[bq, n] = attn_out @ w_o + b_o ---
        for qt in range(BQ // 128):
            for nh in range(D // 512):
                p = ps_b.tile([128, 512], F32, tag="ps_b")
                for c in range(DC):
                    nc.tensor.matmul(p, lhsT=aT[:, c, qt * 128:qt * 128 + 128],
                                     rhs=wo_sb[:, c, nh * 512:(nh + 1) * 512],
                                     start=(c == 0), stop=(c == DC - 1))
                osb = opool.tile([128, 512], F32, tag="osb")
                nc.vector.tensor_add(osb, p, bo_sb[:, nh * 512:(nh + 1) * 512])
                nc.sync.dma_start(
                    out=out[qt * 128:qt * 128 + 128, nh * 512:(nh + 1) * 512],
                    in_=osb)

    nc.compile()
    return nc


_NC_CACHE = None


def _get_program():
    global _NC_CACHE
    if _NC_CACHE is None:
        _NC_CACHE = build_program()
    return _NC_CACHE


def _make_in_maps(x, w_qkv, b_qkv, w_o, b_o):
    x = np.asarray(x, dtype=np.float32).reshape(B * S, D)
    xt = np.ascontiguousarray(x.T).astype(BF16_NP)
    w_qkv = np.asarray(w_qkv, dtype=np.float32)
    b_qkv = np.asarray(b_qkv, dtype=np.float32)
    wo_bf = np.ascontiguousarray(np.asarray(w_o, dtype=np.float32)).astype(BF16_NP)
    b_o = np.asarray(b_o, dtype=np.float32).reshape(1, D)
    in_maps = []
    for c in range(N_CORES):
        lo = c * HC
        hi = lo + HC
        in_maps.append({
            "xt": xt,
            "wq": np.ascontiguousarray(w_qkv[:, lo:hi]).astype(BF16_NP),
            "wk": np.ascontiguousarray(w_qkv[:, D + lo:D + hi]).astype(BF16_NP),
            "wv": np.ascontiguousarray(w_qkv[:, 2 * D + lo:2 * D + hi]).astype(BF16_NP),
            "bq": np.ascontiguousarray(b_qkv[lo:hi].reshape(HC, 1)),
            "bk": np.ascontiguousarray(b_qkv[D + lo:D + hi].reshape(HC, 1)),
            "bv": np.ascontiguousarray(b_qkv[2 * D + lo:2 * D + hi].reshape(HC, 1)),
            "wo": wo_bf,
            "bo": b_o,
        })
    return in_maps


def _assemble(results):
    out = np.empty((B, S, D), dtype=np.float32)
    for c in range(N_CORES):
        q0 = c * 256
        for k in range(B):
            out[k, q0:q0 + 256, :] = results[c]["out"][k * 256:(k + 1) * 256]
    return out


def run(x, mask, w_qkv, b_qkv, w_o, b_o, trace=False, **trace_kwargs):
    """Run on hardware; returns (output, BassKernelResults)."""
    nc = _get_program()
    in_maps = _make_in_maps(x, w_qkv, b_qkv, w_o, b_o)
    res = run_bass_kernel_spmd(nc, in_maps, list(range(N_CORES)),
                               trace=trace, **trace_kwargs)
    return _assemble(res.results), res


def kernel(x, mask, w_qkv, b_qkv, w_o, b_o):
    out, _ = run(x, mask, w_qkv, b_qkv, w_o, b_o)
    return out


# revision 14
# speedup vs baseline: 1.1274x; 1.1274x over previous
"""MultiHeadAttention (B=4, S=2048, D=1024, H=16, causal) on 8 TRN2 NeuronCores.

Sharding: tensor-parallel over heads across all 8 cores (2 heads/core, all 4
batches processed locally; identical SPMD control flow on every core). After
attention, two 8-core AllToAlls (batches 0-1, then 2-3, the first overlapped
with compute) redistribute the transposed attention outputs so each core runs
the output projection for 1/8 of the (batch, seq) rows. Host side only
slices/transposes/casts inputs and concatenates outputs.

Per-core pipeline (all matmuls bf16 with f32 PSUM accumulation):
  - x arrives host-transposed as x^T [D, B*S] in bf16; K^T, Q^T, V^T
    ([head-cols, seq]) via w-stationary matmuls with per-partition bias on the
    ScalarE eviction; V^T is PE-transposed to natural V with a ones column
    appended per head so the PV matmul also produces the softmax denominator.
  - Scores are computed transposed ([k, q] = K @ Q^T) in 512-wide chunks
    aligned to absolute q columns, exp on ScalarE with the 1/sqrt(dk) scale
    folded in (no max subtraction needed: |scores| <~ 2), causal mask applied
    as a 0/1 upper-triangular multiply on diagonal tiles only; fully-masked
    tiles are skipped entirely.
  - PV: [V|1] is the stationary operand (one weight load per k-strip), exp
    chunks stream through, accumulating out^T [65, q] in PSUM; PV emission
    runs one k-strip behind scores/exp so the in-order PE queue never
    head-of-line blocks on ScalarE.
  - out^T is transposed back per q-tile and normalized by the ones column
    ([p,1] broadcast) into the concatenated attention output.
"""

import sys

if "/opt/trn_rl_repo" not in sys.path:
    sys.path.insert(0, "/opt/trn_rl_repo")

from contextlib import ExitStack

import ml_dtypes
import numpy as np

import concourse.bacc as bacc
import concourse.bass as bass
import concourse.mybir as mybir
import concourse.tile as tile
from concourse.bass_utils import run_bass_kernel_spmd
from concourse.masks import make_identity, make_upper_triangular

N_CORES = 8
B = 4
S = 2048
D = 1024
H_TOT = 16
DK = 64
H_LOC = H_TOT // N_CORES  # 2 heads per core
HC = H_LOC * DK  # 128 head-cols per core
ST = S // 128  # 16 seq tiles per batch
DC = D // 128  # 8 d_model chunks
BQ = (B * S) // N_CORES  # 1024 (batch,seq) rows per core after AllToAll

F32 = mybir.dt.float32
BF16 = mybir.dt.bfloat16
BF16_NP = ml_dtypes.bfloat16


def _bcast(handle, rows, cols):
    """AP reading a [1, cols] DRAM tensor broadcast over `rows` partitions."""
    return bass.AP(tensor=handle, offset=0, ap=[[0, rows], [1, cols]])


def build_program():
    nc = bacc.Bacc("TRN2", target_bir_lowering=False, debug=False,
                   num_devices=N_CORES)

    xt = nc.declare_dram_parameter("xt", [D, B * S], BF16, isOutput=False)
    wq = nc.declare_dram_parameter("wq", [D, HC], BF16, isOutput=False)
    wk = nc.declare_dram_parameter("wk", [D, HC], BF16, isOutput=False)
    wv = nc.declare_dram_parameter("wv", [D, HC], BF16, isOutput=False)
    bq = nc.declare_dram_parameter("bq", [HC, 1], F32, isOutput=False)
    bk = nc.declare_dram_parameter("bk", [HC, 1], F32, isOutput=False)
    bv = nc.declare_dram_parameter("bv", [HC, 1], F32, isOutput=False)
    wo = nc.declare_dram_parameter("wo", [D, D], BF16, isOutput=False)
    bo = nc.declare_dram_parameter("bo", [1, D], F32, isOutput=False)
    out = nc.declare_dram_parameter("out", [BQ, D], F32, isOutput=True)

    with ExitStack() as ctx:
        tc = ctx.enter_context(tile.TileContext(nc))

        consts = ctx.enter_context(tc.tile_pool(name="consts", bufs=1))
        wpool = ctx.enter_context(tc.tile_pool(name="wpool", bufs=1))
        xtp = ctx.enter_context(tc.tile_pool(name="xtp", bufs=2))
        kqv = ctx.enter_context(tc.tile_pool(name="kqv", bufs=2))
        epool = ctx.enter_context(tc.tile_pool(name="epool", bufs=8))
        aopool = ctx.enter_context(tc.tile_pool(name="aopool", bufs=2))
        rpool = ctx.enter_context(tc.tile_pool(name="rpool", bufs=4))
        opool = ctx.enter_context(tc.tile_pool(name="opool", bufs=2))
        ps_s = ctx.enter_context(tc.tile_pool(name="ps_s", bufs=1, space="PSUM"))
        ps_b = ctx.enter_context(tc.tile_pool(name="ps_b", bufs=3, space="PSUM"))
        ps_o = ctx.enter_context(tc.tile_pool(name="ps_o", bufs=1, space="PSUM"))
        dram = ctx.enter_context(tc.tile_pool(name="dram", bufs=1, space="DRAM"))

        # four quarter-exchanges (one per batch); only the last is exposed
        in_b = [dram.tile([N_CORES * 128, 256], BF16, tag=f"in_b{i}",
                          name=f"in_b{i}") for i in range(B)]
        out_b = [dram.tile([N_CORES * 128, 256], BF16, tag=f"out_b{i}",
                           name=f"out_b{i}") for i in range(B)]

        # --- constants ---
        ident = consts.tile([128, 128], F32)
        make_identity(nc, ident)
        ident_bf = consts.tile([128, 128], BF16)
        make_identity(nc, ident_bf)
        triu = consts.tile([128, 128], BF16)
        make_upper_triangular(nc, triu, 1.0, diag=True)
        bq_sb = consts.tile([HC, 1], F32)
        nc.sync.dma_start(out=bq_sb, in_=bq[:, :])
        bk_sb = consts.tile([HC, 1], F32)
        nc.sync.dma_start(out=bk_sb, in_=bk[:, :])
        bv_sb = consts.tile([HC, 1], F32)
        nc.sync.dma_start(out=bv_sb, in_=bv[:, :])
        bo_sb = consts.tile([128, D], F32)
        nc.sync.dma_start(out=bo_sb, in_=_bcast(bo, 128, D))

        # --- weights (already bf16) ---
        wq_sb = wpool.tile([128, DC, HC], BF16, tag="wq_sb")
        nc.sync.dma_start(out=wq_sb, in_=wq.rearrange("(c p) m -> p c m", p=128))
        wk_sb = wpool.tile([128, DC, HC], BF16, tag="wk_sb")
        nc.sync.dma_start(out=wk_sb, in_=wk.rearrange("(c p) m -> p c m", p=128))
        wv_sb = wpool.tile([128, DC, HC], BF16, tag="wv_sb")
        nc.sync.dma_start(out=wv_sb, in_=wv.rearrange("(c p) m -> p c m", p=128))
        wo_sb = wpool.tile([128, DC, D], BF16, tag="wo_sb")
        nc.sync.dma_start(out=wo_sb, in_=wo.rearrange("(c p) m -> p c m", p=128))

        def emit_xt_dma(b):
            xT = xtp.tile([128, DC, S], BF16, tag="xT", name=f"xT_{b}")
            for c in range(DC):
                nc.sync.dma_start(
                    out=xT[:, c, :],
                    in_=xt[c * 128:(c + 1) * 128, b * S:(b + 1) * S])
            return xT

        def proj_steps(b, xT):
            """Generator: K^T/Q^T/V^T projection + V transpose for batch b,
            yielded in ~28 PE-dense steps so attention(b-1) emission can
            interleave them (keeps the PE warm through ACT-paced phases)."""
            kt = kqv.tile([HC, S], BF16, tag="kt", name=f"kt_{b}")
            qt_ = kqv.tile([HC, S], BF16, tag="qt", name=f"qt_{b}")
            vt = kqv.tile([HC, S], BF16, tag="vt", name=f"vt_{b}")
            for dst, w_sb, b_sb in ((kt, wk_sb, bk_sb), (qt_, wq_sb, bq_sb),
                                    (vt, wv_sb, bv_sb)):
                for s4 in range(S // 512):
                    p = ps_b.tile([128, 512], F32, tag="ps_b")
                    for c in range(DC):
                        nc.tensor.matmul(p, lhsT=w_sb[:, c, :],
                                         rhs=xT[:, c, s4 * 512:(s4 + 1) * 512],
                                         start=(c == 0), stop=(c == DC - 1))
                    nc.vector.tensor_scalar_add(
                        dst[:, s4 * 512:(s4 + 1) * 512], p, b_sb)
                    yield None
            # V natural [seq, head, 64|1] via PE transpose of V^T
            vsb = kqv.tile([128, ST, H_LOC * 65], BF16, tag="vsb",
                           name=f"vsb_{b}")
            ones_view = vsb.rearrange("p s (h o) -> p s h o", o=65)[:, :, :, 64:65]
            nc.vector.memset(ones_view, 1.0)
            for st in range(ST):
                pt = ps_s.tile([128, 128], BF16, tag="ps_t")
                nc.tensor.transpose(pt, vt[:, st * 128:st * 128 + 128], ident_bf)
                v_view = vsb.rearrange("p s (h o) -> p s h o", o=65)[:, st, :, 0:64]
                nc.vector.tensor_copy(v_view,
                                      pt.rearrange("p (h d) -> p h d", d=DK))
                if st % 2 == 1:
                    yield None
            kqv_tiles[b] = (kt, qt_, vsb)

        def emit_attention(b, interleave):
            kt, qt_, vsb = kqv_tiles[b]
            aos = aopool.tile([128, ST, HC], F32, tag="ao", name=f"aos_{b}")
            for h in range(H_LOC):
                # out^T accumulator: [65, q]; each 512-col bank holds 4 q-tiles
                po = ps_o.tile([65, ST * 128], F32, tag="ps_o",
                               name=f"po_{b}_{h}")

                def emit_pv(chunks):
                    # PV one strip behind scores/exp: [V|1] stationary (one
                    # LDWEIGHTS per strip), exp chunks stream as the moving
                    # operand. start clears has_written for the whole PSUM
                    # bank, so only the j==0 chunks (each covering exactly
                    # one bank) set it.
                    for ec, j0, c0, cw in chunks:
                        a = c0 // 512
                        nc.tensor.matmul(
                            po[:, c0:c0 + cw],
                            lhsT=vsb[:, j0, h * 65:(h + 1) * 65],
                            rhs=ec[:, 0:cw],
                            start=(j0 == 0), stop=(j0 == 4 * a + 3),
                            skip_group_check=True)

                prev = None
                for j in range(ST):
                    cur = []
                    for a in range(j // 4, 4):
                        c0 = max(512 * a, 128 * j)
                        cw = 512 * (a + 1) - c0
                        ps = ps_b.tile([128, 512], F32, tag="ps_b")
                        nc.tensor.matmul(
                            ps[:, 0:cw],
                            lhsT=kt[h * DK:(h + 1) * DK, j * 128:j * 128 + 128],
                            rhs=qt_[h * DK:(h + 1) * DK, c0:c0 + cw],
                            start=True, stop=True)
                        ec = epool.tile([128, 512], BF16, tag="et")
                        nc.scalar.activation(ec[:, 0:cw], ps[:, 0:cw],
                                             mybir.ActivationFunctionType.Exp,
                                             scale=1.0 / np.sqrt(DK))
                        if c0 == 128 * j:
                            # first 128 cols of the strip are the diagonal
                            nc.vector.tensor_mul(ec[:, 0:128], ec[:, 0:128],
                                                 triu)
                        cur.append((ec, j, c0, cw))
                    if prev:
                        emit_pv(prev)
                    prev = cur
                    next(interleave, None)
                emit_pv(prev)

                # evict out^T, transpose each q-tile back, normalize
                poT = aopool.tile([65, ST * 128], F32, tag="poT",
                                  name=f"poT_{b}_{h}")
                nc.vector.tensor_copy(poT, po)
                for g in range(ST):
                    pt = ps_s.tile([128, 128], F32, tag="ps_t")
                    nc.tensor.transpose(pt[:, 0:65],
                                        poT[:, g * 128:g * 128 + 128],
                                        ident[0:65, 0:65])
                    rcp = rpool.tile([128, 1], F32, tag="rcp")
                    nc.vector.reciprocal(rcp, pt[:, 64:65])
                    nc.vector.tensor_scalar_mul(
                        aos[:, g, h * DK:(h + 1) * DK], pt[:, 0:64], rcp)
                    if g % 4 == 3:
                        next(interleave, None)

            # transpose [q, dm] -> [dm, q], cast bf16, ship to bounce buffer
            for g in range(ST):
                pt = ps_s.tile([128, 128], F32, tag="ps_t")
                nc.tensor.transpose(pt, aos[:, g, :], ident)
                aoT = aopool.tile([128, 128], BF16, tag="aoT")
                nc.vector.tensor_copy(aoT, pt)
                shard = g // 2
                col = (g % 2) * 128
                nc.sync.dma_start(
                    out=in_b[b][shard * 128:(shard + 1) * 128, col:col + 128],
                    in_=aoT)
                if g % 4 == 3:
                    next(interleave, None)

            nc.gpsimd.collective_compute(
                "AllToAll", mybir.AluOpType.bypass,
                replica_groups=[list(range(N_CORES))],
                ins=[in_b[b].opt()], outs=[out_b[b].opt()])

        kqv_tiles = {}
        xT0 = emit_xt_dma(0)
        for _ in proj_steps(0, xT0):
            pass
        for b in range(B):
            if b + 1 < B:
                xTn = emit_xt_dma(b + 1)
                nxt = proj_steps(b + 1, xTn)
            else:
                nxt = iter(())
            emit_attention(b, nxt)
            for _ in nxt:
                pass

        # full attn_out^T for my 1/8 of (b, q): rows = my two half-batches
        aT = wpool.tile([128, DC, BQ], BF16, tag="aT")
        for c in range(DC):
            for k in range(B):
                nc.sync.dma_start(
                    out=aT[:, c, k * 256:(k + 1) * 256],
                    in_=out_b[k][c * 128:(c + 1) * 128, :])

        # --- output projection: out[bq, n] = attn_out @ w_o + b_o ---
        for qt in range(BQ // 128):
            for nh in range(D // 512):
                p = ps_b.tile([128, 512], F32, tag="ps_b")
                for c in range(DC):
                    nc.tensor.matmul(p, lhsT=aT[:, c, qt * 128:qt * 128 + 128],
                                     rhs=wo_sb[:, c, nh * 512:(nh + 1) * 512],
                                     start=(c == 0), stop=(c == DC - 1))
                osb = opool.tile([128, 512], F32, tag="osb")
                nc.vector.tensor_add(osb, p, bo_sb[:, nh * 512:(nh + 1) * 512])
                nc.sync.dma_start(
                    out=out[qt * 128:qt * 128 + 128, nh * 512:(nh + 1) * 512],
                    in_=osb)

    nc.compile()
    return nc


_NC_CACHE = None


def _get_program():
    global _NC_CACHE
    if _NC_CACHE is None:
        _NC_CACHE = build_program()
    return _NC_CACHE


def _make_in_maps(x, w_qkv, b_qkv, w_o, b_o):
    x = np.asarray(x, dtype=np.float32).reshape(B * S, D)
    xt = np.ascontiguousarray(x.T).astype(BF16_NP)
    w_qkv = np.asarray(w_qkv, dtype=np.float32)
    b_qkv = np.asarray(b_qkv, dtype=np.float32)
    wo_bf = np.ascontiguousarray(np.asarray(w_o, dtype=np.float32)).astype(BF16_NP)
    b_o = np.asarray(b_o, dtype=np.float32).reshape(1, D)
    in_maps = []
    for c in range(N_CORES):
        lo = c * HC
        hi = lo + HC
        in_maps.append({
            "xt": xt,
            "wq": np.ascontiguousarray(w_qkv[:, lo:hi]).astype(BF16_NP),
            "wk": np.ascontiguousarray(w_qkv[:, D + lo:D + hi]).astype(BF16_NP),
            "wv": np.ascontiguousarray(w_qkv[:, 2 * D + lo:2 * D + hi]).astype(BF16_NP),
            "bq": np.ascontiguousarray(b_qkv[lo:hi].reshape(HC, 1)),
            "bk": np.ascontiguousarray(b_qkv[D + lo:D + hi].reshape(HC, 1)),
            "bv": np.ascontiguousarray(b_qkv[2 * D + lo:2 * D + hi].reshape(HC, 1)),
            "wo": wo_bf,
            "bo": b_o,
        })
    return in_maps


def _assemble(results):
    out = np.empty((B, S, D), dtype=np.float32)
    for c in range(N_CORES):
        q0 = c * 256
        for k in range(B):
            out[k, q0:q0 + 256, :] = results[c]["out"][k * 256:(k + 1) * 256]
    return out


def run(x, mask, w_qkv, b_qkv, w_o, b_o, trace=False, **trace_kwargs):
    """Run on hardware; returns (output, BassKernelResults)."""
    nc = _get_program()
    in_maps = _make_in_maps(x, w_qkv, b_qkv, w_o, b_o)
    res = run_bass_kernel_spmd(nc, in_maps, list(range(N_CORES)),
                               trace=trace, **trace_kwargs)
    return _assemble(res.results), res


def kernel(x, mask, w_qkv, b_qkv, w_o, b_o):
    out, _ = run(x, mask, w_qkv, b_qkv, w_o, b_o)
    return out


# revision 15
# speedup vs baseline: 1.1990x; 1.0636x over previous
"""MultiHeadAttention (B=4, S=2048, D=1024, H=16, causal) on 8 TRN2 NeuronCores.

Sharding: tensor-parallel over heads across all 8 cores (2 heads/core, all 4
batches processed locally; identical SPMD control flow on every core). After
attention, two 8-core AllToAlls (batches 0-1, then 2-3, the first overlapped
with compute) redistribute the transposed attention outputs so each core runs
the output projection for 1/8 of the (batch, seq) rows. Host side only
slices/transposes/casts inputs and concatenates outputs.

Per-core pipeline (all matmuls bf16 with f32 PSUM accumulation):
  - x arrives host-transposed as x^T [D, B*S] in bf16; K^T, Q^T, V^T
    ([head-cols, seq]) via w-stationary matmuls with per-partition bias on the
    ScalarE eviction; V^T is PE-transposed to natural V with a ones column
    appended per head so the PV matmul also produces the softmax denominator.
  - Scores are computed transposed ([k, q] = K @ Q^T) in 512-wide chunks
    aligned to absolute q columns, exp on ScalarE with the 1/sqrt(dk) scale
    folded in (no max subtraction needed: |scores| <~ 2), causal mask applied
    as a 0/1 upper-triangular multiply on diagonal tiles only; fully-masked
    tiles are skipped entirely.
  - PV: [V|1] is the stationary operand (one weight load per k-strip), exp
    chunks stream through, accumulating out^T [65, q] in PSUM; PV emission
    runs one k-strip behind scores/exp so the in-order PE queue never
    head-of-line blocks on ScalarE.
  - out^T is transposed back per q-tile and normalized by the ones column
    ([p,1] broadcast) into the concatenated attention output.
"""

import sys

if "/opt/trn_rl_repo" not in sys.path:
    sys.path.insert(0, "/opt/trn_rl_repo")

from contextlib import ExitStack

import ml_dtypes
import numpy as np

import concourse.bacc as bacc
import concourse.bass as bass
import concourse.mybir as mybir
import concourse.tile as tile
from concourse.bass_utils import run_bass_kernel_spmd
from concourse.masks import make_identity, make_upper_triangular

N_CORES = 8
B = 4
S = 2048
D = 1024
H_TOT = 16
DK = 64
H_LOC = H_TOT // N_CORES  # 2 heads per core
HC = H_LOC * DK  # 128 head-cols per core
ST = S // 128  # 16 seq tiles per batch
DC = D // 128  # 8 d_model chunks
BQ = (B * S) // N_CORES  # 1024 (batch,seq) rows per core after AllToAll

F32 = mybir.dt.float32
BF16 = mybir.dt.bfloat16
BF16_NP = ml_dtypes.bfloat16


def _bcast(handle, rows, cols):
    """AP reading a [1, cols] DRAM tensor broadcast over `rows` partitions."""
    return bass.AP(tensor=handle, offset=0, ap=[[0, rows], [1, cols]])


def build_program():
    nc = bacc.Bacc("TRN2", target_bir_lowering=False, debug=False,
                   num_devices=N_CORES)

    xt = nc.declare_dram_parameter("xt", [D, B * S], BF16, isOutput=False)
    wq = nc.declare_dram_parameter("wq", [D, HC], BF16, isOutput=False)
    wk = nc.declare_dram_parameter("wk", [D, HC], BF16, isOutput=False)
    wv = nc.declare_dram_parameter("wv", [D, HC], BF16, isOutput=False)
    bq = nc.declare_dram_parameter("bq", [HC, 1], F32, isOutput=False)
    bk = nc.declare_dram_parameter("bk", [HC, 1], F32, isOutput=False)
    bv = nc.declare_dram_parameter("bv", [HC, 1], F32, isOutput=False)
    wo = nc.declare_dram_parameter("wo", [D, D], BF16, isOutput=False)
    bo = nc.declare_dram_parameter("bo", [1, D], F32, isOutput=False)
    out = nc.declare_dram_parameter("out", [BQ, D], F32, isOutput=True)

    with ExitStack() as ctx:
        tc = ctx.enter_context(tile.TileContext(nc))

        consts = ctx.enter_context(tc.tile_pool(name="consts", bufs=1))
        wpool = ctx.enter_context(tc.tile_pool(name="wpool", bufs=1))
        xtp = ctx.enter_context(tc.tile_pool(name="xtp", bufs=2))
        kqv = ctx.enter_context(tc.tile_pool(name="kqv", bufs=2))
        epool = ctx.enter_context(tc.tile_pool(name="epool", bufs=4))
        aopool = ctx.enter_context(tc.tile_pool(name="aopool", bufs=2))
        rpool = ctx.enter_context(tc.tile_pool(name="rpool", bufs=4))
        opool = ctx.enter_context(tc.tile_pool(name="opool", bufs=2))
        ps_s = ctx.enter_context(tc.tile_pool(name="ps_s", bufs=2, space="PSUM"))
        ps_b = ctx.enter_context(tc.tile_pool(name="ps_b", bufs=2, space="PSUM"))
        ps_o = ctx.enter_context(tc.tile_pool(name="ps_o", bufs=1, space="PSUM"))
        dram = ctx.enter_context(tc.tile_pool(name="dram", bufs=1, space="DRAM"))

        # four quarter-exchanges (one per batch); only the last is exposed
        in_b = [dram.tile([N_CORES * 128, 256], BF16, tag=f"in_b{i}",
                          name=f"in_b{i}") for i in range(B)]
        out_b = [dram.tile([N_CORES * 128, 256], BF16, tag=f"out_b{i}",
                           name=f"out_b{i}") for i in range(B)]

        # --- constants ---
        ident = consts.tile([128, 128], F32)
        make_identity(nc, ident)
        ident_bf = consts.tile([128, 128], BF16)
        make_identity(nc, ident_bf)
        triu = consts.tile([128, 128], BF16)
        make_upper_triangular(nc, triu, 1.0, diag=True)
        bq_sb = consts.tile([HC, 1], F32)
        nc.sync.dma_start(out=bq_sb, in_=bq[:, :])
        bk_sb = consts.tile([HC, 1], F32)
        nc.sync.dma_start(out=bk_sb, in_=bk[:, :])
        bv_sb = consts.tile([HC, 1], F32)
        nc.sync.dma_start(out=bv_sb, in_=bv[:, :])
        bo_sb = consts.tile([128, D], F32)
        nc.sync.dma_start(out=bo_sb, in_=_bcast(bo, 128, D))

        # --- weights (already bf16) ---
        wq_sb = wpool.tile([128, DC, HC], BF16, tag="wq_sb")
        nc.sync.dma_start(out=wq_sb, in_=wq.rearrange("(c p) m -> p c m", p=128))
        wk_sb = wpool.tile([128, DC, HC], BF16, tag="wk_sb")
        nc.sync.dma_start(out=wk_sb, in_=wk.rearrange("(c p) m -> p c m", p=128))
        wv_sb = wpool.tile([128, DC, HC], BF16, tag="wv_sb")
        nc.sync.dma_start(out=wv_sb, in_=wv.rearrange("(c p) m -> p c m", p=128))
        wo_sb = wpool.tile([128, DC, D], BF16, tag="wo_sb")
        nc.sync.dma_start(out=wo_sb, in_=wo.rearrange("(c p) m -> p c m", p=128))

        def emit_xt_dma(b):
            xT = xtp.tile([128, DC, S], BF16, tag="xT", name=f"xT_{b}")
            for c in range(DC):
                nc.sync.dma_start(
                    out=xT[:, c, :],
                    in_=xt[c * 128:(c + 1) * 128, b * S:(b + 1) * S])
            return xT

        def proj_steps(b, xT):
            """Generator: K^T/Q^T/V^T projection + V transpose for batch b,
            yielded in ~28 PE-dense steps so attention(b-1) emission can
            interleave them (keeps the PE warm through ACT-paced phases)."""
            kt = kqv.tile([HC, S], BF16, tag="kt", name=f"kt_{b}")
            qt_ = kqv.tile([HC, S], BF16, tag="qt", name=f"qt_{b}")
            vt = kqv.tile([HC, S], BF16, tag="vt", name=f"vt_{b}")
            for dst, w_sb, b_sb in ((kt, wk_sb, bk_sb), (qt_, wq_sb, bq_sb),
                                    (vt, wv_sb, bv_sb)):
                for s4 in range(S // 512):
                    p = ps_b.tile([128, 512], F32, tag="ps_b")
                    for c in range(DC):
                        nc.tensor.matmul(p, lhsT=w_sb[:, c, :],
                                         rhs=xT[:, c, s4 * 512:(s4 + 1) * 512],
                                         start=(c == 0), stop=(c == DC - 1))
                    nc.vector.tensor_scalar_add(
                        dst[:, s4 * 512:(s4 + 1) * 512], p, b_sb)
                    yield None
            # V natural [seq, head, 64|1] via PE transpose of V^T
            vsb = kqv.tile([128, ST, H_LOC * 65], BF16, tag="vsb",
                           name=f"vsb_{b}")
            ones_view = vsb.rearrange("p s (h o) -> p s h o", o=65)[:, :, :, 64:65]
            nc.vector.memset(ones_view, 1.0)
            for st in range(ST):
                pt = ps_s.tile([128, 128], BF16, tag="ps_t")
                nc.tensor.transpose(pt, vt[:, st * 128:st * 128 + 128], ident_bf)
                v_view = vsb.rearrange("p s (h o) -> p s h o", o=65)[:, st, :, 0:64]
                nc.vector.tensor_copy(v_view,
                                      pt.rearrange("p (h d) -> p h d", d=DK))
                if st % 2 == 1:
                    yield None
            kqv_tiles[b] = (kt, qt_, vsb)

        def emit_attention(b, interleave):
            kt, qt_, vsb = kqv_tiles[b]
            aos = aopool.tile([128, ST, HC], F32, tag="ao", name=f"aos_{b}")
            for h in range(H_LOC):
                # attention in two q-halves so scores PSUM can be 1024 wide
                # (single wide exp per k-strip amortizes ScalarE's ~352-cycle
                # per-ACTIVATE overhead; ScalarE is the co-bottleneck)
                for ha in range(2):
                    q0 = ha * 1024
                    po = ps_o.tile([65, 1024], F32, tag="ps_o",
                                   name=f"po_{b}_{h}_{ha}")

                    def emit_pv(pend, ha=ha, po=po):
                        # PV one strip behind scores/exp: [V|1] stationary,
                        # exp chunks stream. start clears has_written for the
                        # whole PSUM bank, so only j==0 (covering each bank
                        # fully) sets it; stop on each bank's last strip.
                        es_t, j0, rel0 = pend
                        for a in range(2):
                            lo = max(512 * a, rel0)
                            hi = 512 * (a + 1)
                            if lo >= hi:
                                continue
                            nc.tensor.matmul(
                                po[:, lo:hi],
                                lhsT=vsb[:, j0, h * 65:(h + 1) * 65],
                                rhs=es_t[:, lo:hi],
                                start=(j0 == 0),
                                stop=(j0 == 8 * ha + 4 * a + 3),
                                skip_group_check=True)

                    prev = None
                    for j in range(8 * ha + 8):
                        rel0 = max(0, j * 128 - q0)
                        ps = ps_b.tile([128, 1024], F32, tag="ps_b")
                        for a in range(2):
                            lo = max(512 * a, rel0)
                            hi = 512 * (a + 1)
                            if lo >= hi:
                                continue
                            nc.tensor.matmul(
                                ps[:, lo:hi],
                                lhsT=kt[h * DK:(h + 1) * DK,
                                        j * 128:j * 128 + 128],
                                rhs=qt_[h * DK:(h + 1) * DK,
                                        q0 + lo:q0 + hi],
                                start=True, stop=True)
                        es_t = epool.tile([128, 1024], BF16, tag="et")
                        nc.scalar.activation(es_t[:, rel0:1024],
                                             ps[:, rel0:1024],
                                             mybir.ActivationFunctionType.Exp,
                                             scale=1.0 / np.sqrt(DK))
                        if j * 128 >= q0:
                            # diagonal tile lives in this half
                            nc.vector.tensor_mul(es_t[:, rel0:rel0 + 128],
                                                 es_t[:, rel0:rel0 + 128],
                                                 triu)
                        if prev:
                            emit_pv(prev)
                        prev = (es_t, j, rel0)
                        next(interleave, None)
                    emit_pv(prev)

                    # evict out^T, transpose each q-tile back, normalize
                    poT = aopool.tile([65, 1024], F32, tag="poT",
                                      name=f"poT_{b}_{h}_{ha}")
                    nc.vector.tensor_copy(poT, po)
                    for gr in range(8):
                        g = ha * 8 + gr
                        pt = ps_s.tile([128, 128], F32, tag="ps_t")
                        nc.tensor.transpose(pt[:, 0:65],
                                            poT[:, gr * 128:gr * 128 + 128],
                                            ident[0:65, 0:65])
                        rcp = rpool.tile([128, 1], F32, tag="rcp")
                        nc.vector.reciprocal(rcp, pt[:, 64:65])
                        nc.vector.tensor_scalar_mul(
                            aos[:, g, h * DK:(h + 1) * DK], pt[:, 0:64], rcp)
                        if gr % 4 == 3:
                            next(interleave, None)

            # transpose [q, dm] -> [dm, q], cast bf16, ship to bounce buffer
            for g in range(ST):
                pt = ps_s.tile([128, 128], F32, tag="ps_t")
                nc.tensor.transpose(pt, aos[:, g, :], ident)
                aoT = aopool.tile([128, 128], BF16, tag="aoT")
                nc.vector.tensor_copy(aoT, pt)
                shard = g // 2
                col = (g % 2) * 128
                nc.sync.dma_start(
                    out=in_b[b][shard * 128:(shard + 1) * 128, col:col + 128],
                    in_=aoT)
                if g % 4 == 3:
                    next(interleave, None)

            nc.gpsimd.collective_compute(
                "AllToAll", mybir.AluOpType.bypass,
                replica_groups=[list(range(N_CORES))],
                ins=[in_b[b].opt()], outs=[out_b[b].opt()])

        kqv_tiles = {}
        xT0 = emit_xt_dma(0)
        for _ in proj_steps(0, xT0):
            pass
        for b in range(B):
            if b + 1 < B:
                xTn = emit_xt_dma(b + 1)
                nxt = proj_steps(b + 1, xTn)
            else:
                nxt = iter(())
            emit_attention(b, nxt)
            for _ in nxt:
                pass

        # full attn_out^T for my 1/8 of (b, q): rows = my two half-batches
        aT = wpool.tile([128, DC, BQ], BF16, tag="aT")
        for c in range(DC):
            for k in range(B):
                nc.sync.dma_start(
                    out=aT[:, c, k * 256:(k + 1) * 256],
                    in_=out_b[k][c * 128:(c + 1) * 128, :])

        # --- output projection: out[bq, n] = attn_out @ w_o + b_o ---
        for qt in range(BQ // 128):
            for nh in range(D // 512):
                p = ps_b.tile([128, 512], F32, tag="ps_b")
                for c in range(DC):
                    nc.tensor.matmul(p, lhsT=aT[:, c, qt * 128:qt * 128 + 128],
                                     rhs=wo_sb[:, c, nh * 512:(nh + 1) * 512],
                                     start=(c == 0), stop=(c == DC - 1))
                osb = opool.tile([128, 512], F32, tag="osb")
                nc.vector.tensor_add(osb, p, bo_sb[:, nh * 512:(nh + 1) * 512])
                nc.sync.dma_start(
                    out=out[qt * 128:qt * 128 + 128, nh * 512:(nh + 1) * 512],
                    in_=osb)

    nc.compile()
    return nc


_NC_CACHE = None


def _get_program():
    global _NC_CACHE
    if _NC_CACHE is None:
        _NC_CACHE = build_program()
    return _NC_CACHE


def _make_in_maps(x, w_qkv, b_qkv, w_o, b_o):
    x = np.asarray(x, dtype=np.float32).reshape(B * S, D)
    xt = np.ascontiguousarray(x.T).astype(BF16_NP)
    w_qkv = np.asarray(w_qkv, dtype=np.float32)
    b_qkv = np.asarray(b_qkv, dtype=np.float32)
    wo_bf = np.ascontiguousarray(np.asarray(w_o, dtype=np.float32)).astype(BF16_NP)
    b_o = np.asarray(b_o, dtype=np.float32).reshape(1, D)
    in_maps = []
    for c in range(N_CORES):
        lo = c * HC
        hi = lo + HC
        in_maps.append({
            "xt": xt,
            "wq": np.ascontiguousarray(w_qkv[:, lo:hi]).astype(BF16_NP),
            "wk": np.ascontiguousarray(w_qkv[:, D + lo:D + hi]).astype(BF16_NP),
            "wv": np.ascontiguousarray(w_qkv[:, 2 * D + lo:2 * D + hi]).astype(BF16_NP),
            "bq": np.ascontiguousarray(b_qkv[lo:hi].reshape(HC, 1)),
            "bk": np.ascontiguousarray(b_qkv[D + lo:D + hi].reshape(HC, 1)),
            "bv": np.ascontiguousarray(b_qkv[2 * D + lo:2 * D + hi].reshape(HC, 1)),
            "wo": wo_bf,
            "bo": b_o,
        })
    return in_maps


def _assemble(results):
    out = np.empty((B, S, D), dtype=np.float32)
    for c in range(N_CORES):
        q0 = c * 256
        for k in range(B):
            out[k, q0:q0 + 256, :] = results[c]["out"][k * 256:(k + 1) * 256]
    return out


def run(x, mask, w_qkv, b_qkv, w_o, b_o, trace=False, **trace_kwargs):
    """Run on hardware; returns (output, BassKernelResults)."""
    nc = _get_program()
    in_maps = _make_in_maps(x, w_qkv, b_qkv, w_o, b_o)
    res = run_bass_kernel_spmd(nc, in_maps, list(range(N_CORES)),
                               trace=trace, **trace_kwargs)
    return _assemble(res.results), res


def kernel(x, mask, w_qkv, b_qkv, w_o, b_o):
    out, _ = run(x, mask, w_qkv, b_qkv, w_o, b_o)
    return out
